# revision 68
# baseline (speedup 1.0000x reference)
"""Self-contained Trainium2 Bass kernel for causal multi-head attention.

Problem: B=2, S=2048, D=1024, H=16 heads (dk=64), fp32, causal + padding mask.
Sharding across 8 NeuronCores: core c -> batch c//4, head-group c%4 (4 heads).

v2 design (all-bf16 dataflow, dense PE schedule):
  - Every matmul operand is bf16 (PSUM accumulates f32): 1 cycle/column on
    the PE at any width, no fp32r narrow-N penalty, and input DMA halves.
  - Inputs stream as a handful of large DMAs split over the two HWDGE
    queues (sync: xt chunks; scalar: weights) so issue cost stays ~5us.
  - Padding mask is folded into the exp activation as a per-partition bias
    (-60000 at padded keys -> exp == 0), so V needs no zeroing and the
    softmax denominator column in V is constant 1.
  - qT/kT stored transposed [dk, S]; scores computed transposed S_T[k, q].
  - No max-subtraction in softmax (scores are O(+-10); exp cannot overflow).
  - Softmax denominator: appended ones column in V (PV matmul row 64).
  - Causal: additive -8e9 triangle on diagonal 128-blocks (pre-scale).
  - Normalization: reciprocal of the denominator read straight from PSUM,
    cast to bf16, broadcast to the pair's 128 partitions with one K=2
    selector matmul, multiplied into ctx on the PSUM->SBUF pass.  ctx for
    a head PAIR is packed into one 128-partition tile, so the output
    projection needs only 2 accumulation steps (K=128 each, no zero rows).
  - The broadcast matmul and ctx multiply for a pair are deferred into the
    NEXT pair's kb stream so the PE never waits on the DVE chain.
Fully-masked rows (all keys up to q padded) produce NaN/garbage on device
and are overwritten on host with the uniform-attention reference value.
"""

import numpy as np
from contextlib import ExitStack

import concourse.bass as bass
import concourse.bacc as bacc
import concourse.tile as tile
import concourse.mybir as mybir
from concourse.bass import ds, ts

F32 = mybir.dt.float32
BF = mybir.dt.bfloat16
AF = mybir.ActivationFunctionType

P = 128
S = 2048
D = 1024
HL = 4          # heads per core
DK = 64
KT = D // P     # 8 k-tiles over the model dim
ST = S // P     # 16 seq tiles
NQC = 4         # 512-wide query chunks
NEG = -8.0e9    # pre-scale causal mask value; *0.125 -> exp underflows to 0
PADBIAS = -60000.0  # post-scale padding bias inside exp
N_CORES = 8
N_HEAD = 16

VW = HL * (DK + 1) + DK - 1  # 323: per-head 65-wide groups, padded slice room


def build_program(num_devices=N_CORES, dbg=False):
    nc = bacc.Bacc(
        "TRN2",
        target_bir_lowering=False,
        debug=False,
        enable_asserts=True,
        num_devices=num_devices,
    )
    # all bulk inputs pre-swizzled on host into SBUF layout so every DMA
    # line is fully contiguous per partition
    ins = {
        "xt": nc.dram_tensor("xt", [P, NQC, KT, 512], BF, kind="ExternalInput").ap(),
        "wq": nc.dram_tensor("wq", [P, KT, 2 * P], BF, kind="ExternalInput").ap(),
        "wk": nc.dram_tensor("wk", [P, KT, 2 * P], BF, kind="ExternalInput").ap(),
        "wv": nc.dram_tensor("wv", [P, KT, 2 * P], BF, kind="ExternalInput").ap(),
        "wo": nc.dram_tensor("wo", [P, 2, D], BF, kind="ExternalInput").ap(),
        "bq": nc.dram_tensor("bq", [P, 2], F32, kind="ExternalInput").ap(),
        "padneg": nc.dram_tensor("padneg", [P, ST], F32, kind="ExternalInput").ap(),
        "tri": nc.dram_tensor("tri", [P, P], F32, kind="ExternalInput").ap(),
    }
    y = nc.dram_tensor("y", [S, D], BF, kind="ExternalOutput").ap()
    if dbg:
        ins["dbg_vaug"] = nc.dram_tensor(
            "dbg_vaug", [P, VW], BF, kind="ExternalOutput"
        ).ap()
        ins["dbg_rcp"] = nc.dram_tensor(
            "dbg_rcp", [1, 2, 512], F32, kind="ExternalOutput"
        ).ap()
        ins["dbg_ctx"] = nc.dram_tensor(
            "dbg_ctx", [P, 512], BF, kind="ExternalOutput"
        ).ap()
        ins["dbg_den"] = nc.dram_tensor(
            "dbg_den", [1, 2, 512], F32, kind="ExternalOutput"
        ).ap()

    with tile.TileContext(nc) as tc:
        _body(tc, y, ins)

    nc.compile()
    return nc


def _body(tc, y, ins):
    nc = tc.nc

    with ExitStack() as ctx:
        const = ctx.enter_context(tc.tile_pool(name="const", bufs=1))
        pt_pool = ctx.enter_context(tc.tile_pool(name="pt", bufs=3))
        rrp = ctx.enter_context(tc.tile_pool(name="rr", bufs=2))
        ysb = ctx.enter_context(tc.tile_pool(name="ysb", bufs=2))
        psA = ctx.enter_context(tc.tile_pool(name="psA", bufs=2, space="PSUM"))
        psB = ctx.enter_context(tc.tile_pool(name="psB", bufs=2, space="PSUM"))
        psY = ctx.enter_context(tc.tile_pool(name="psY", bufs=2, space="PSUM"))

        # ---------------- input DMAs ----------------
        # sync HWDGE queue: the four 1MB xt chunks (needed in order).
        # scalar HWDGE queue: weights + small constants.  All transfers are
        # contiguous per partition (host pre-swizzled).
        xt_sb = const.tile([P, NQC, KT, 512], BF)
        wq_sb = const.tile([P, KT, 2 * P], BF)
        wk_sb = const.tile([P, KT, 2 * P], BF)
        wv_sb = const.tile([P, KT, 2 * P], BF)
        nc.scalar.dma_start(wq_sb[:], ins["wq"])
        nc.sync.dma_start(xt_sb[:, 0], ins["xt"][:, 0])
        nc.scalar.dma_start(wk_sb[:], ins["wk"])
        nc.scalar.dma_start(wv_sb[:], ins["wv"])
        for n in range(1, 4):
            nc.sync.dma_start(xt_sb[:, n], ins["xt"][:, n])
        # bq is needed by the first q-projection copy, so it goes HWDGE
        bq_sb = const.tile([P, 2], F32)
        nc.scalar.dma_start(bq_sb[:], ins["bq"])
        padneg_sb = const.tile([P, ST], F32)
        nc.gpsimd.dma_start(padneg_sb[:], ins["padneg"])
        tri_sb = const.tile([P, P], F32)
        nc.gpsimd.dma_start(tri_sb[:], ins["tri"])
        # wo packed per head pair: partition r, pair m -> Wo column g*256+m*128+r
        wo_sb = const.tile([P, 2, D], BF)
        nc.gpsimd.dma_start(wo_sb[:], ins["wo"])

        ones_sb = const.tile([1, 512], BF)
        nc.vector.memset(ones_sb[:], 1.0)
        # selectors for the denominator broadcast (partition-0 rows; engine
        # ops may not start at partition 1): selh[0] targets partitions
        # 0-63, selh[1] targets 64-127 via two K=1 accumulating matmuls
        selh = const.tile([1, 2, P], BF)
        nc.vector.memset(selh[:], 0.0)
        nc.vector.memset(selh[:, 0, 0:DK], 1.0)
        nc.vector.memset(selh[:, 1, DK:P], 1.0)

        qt_sb = const.tile([P, 2, S], BF)
        kt_sb = const.tile([P, 2, S], BF)
        # per head: 64 value cols + 1 all-ones denominator col; padded so a
        # 128-wide stationary slice starting at h*65 stays in bounds (the
        # extra columns produce junk output rows 65-127, never read)
        vaug_sb = const.tile([P, ST, VW], BF)
        nc.vector.memset(vaug_sb[:, :, HL * (DK + 1) : VW], 0.0)
        den_cols = vaug_sb[:, :, 0 : HL * (DK + 1)].rearrange(
            "p s (h c) -> p s h c", c=DK + 1
        )[:, :, :, DK : DK + 1]
        nc.vector.memset(den_cols, 1.0)

        # normalized per-PAIR context [h0 dims 0-63 | h1 dims 64-127];
        # one set per query chunk (no reuse), so output-projection filler for
        # chunk qc can run arbitrarily late without WAR pressure
        ctx_sets = [
            [
                const.tile([P, 512], BF, name=f"ctxsb{st}_{m}", tag=f"ctxsb{st}_{m}")
                for m in range(2)
            ]
            for st in range(NQC)
        ]

        # PE warmup while the input DMAs stream (HAM un-throttle needs
        # ~3.4us of sustained matmul activity; these are dep-free)
        warm_ps = psY.tile([P, 512], F32, name="warm", tag="yp")
        for i in range(20):
            nc.tensor.matmul(
                warm_ps[:], ones_sb[:, 0:P], ones_sb[:], start=True, stop=True
            )

        # ---------------- projections for one 512-token chunk ----------------
        # Emitted as self-contained "steps" (~1.7-4us of PE work each) so the
        # schedule can sprinkle them between attention key-blocks.
        def proj_qk_step(n, tgt, w_sb, bias, m):
            def step():
                ps = psA.tile([P, 1024], F32, name=f"ps_p{n}{m}", tag="ps")
                for k in range(KT):
                    nc.tensor.matmul(
                        ps[:, 0:512],
                        w_sb[:, k, ts(m, P)],
                        xt_sb[:, n, k, :],
                        start=(k == 0),
                        stop=(k == KT - 1),
                    )
                out_ap = tgt[:, m, ds(n * 512, 512)]
                if bias is not None:
                    nc.vector.tensor_scalar_add(
                        out_ap, ps[:, 0:512], bias[:, m : m + 1]
                    )
                else:
                    nc.vector.tensor_copy(out_ap, ps[:, 0:512])

            return step

        def proj_v_step(n, si):
            def step():
                s = n * 4 + si
                ps = psA.tile([P, 1024], F32, name=f"ps_v{s}", tag="ps")
                for k in range(KT):
                    nc.tensor.matmul(
                        ps[:, 0:256],
                        xt_sb[:, n, k, ts(si, P)],
                        wv_sb[:, k, :],
                        start=(k == 0),
                        stop=(k == KT - 1),
                    )
                vdst = vaug_sb[:, s, 0 : HL * (DK + 1)].rearrange(
                    "p (h c) -> p h c", c=DK + 1
                )[:, :, 0:DK]
                vsrc = ps[:, 0:256].rearrange("p (h c) -> p h c", c=DK)
                nc.vector.tensor_copy(vdst, vsrc)

            return step

        def proj_steps(n):
            out = []
            for m in range(2):
                out.append(proj_qk_step(n, qt_sb, wq_sb, bq_sb, m))
                out.append(proj_qk_step(n, kt_sb, wk_sb, None, m))
            for si in range(4):
                out.append(proj_v_step(n, si))
            return out

        def proj_chunk(n):
            for st in proj_steps(n):
                st()

        # ---------------- attention for one 512-query chunk ----------------
        y_r = y.rearrange("(t p) n -> t p n", p=P)

        # deferred normalization state: [(qc, m, pvs, ctxtmp, rcp_bf)]
        pending_norm = []

        def start_norm(qc, m, pvs):
            """ctx copies to SBUF + reciprocal of the denominator rows.
            The ctx copies free the pair's PSUM banks (which gate the next
            pair's PV), so they come first — except for the final pair,
            where the reciprocal chain (gating the tail broadcast matmul)
            gets priority.  The denominator rows bounce through SBUF: DVE
            reciprocal_approx_fast reads garbage from PSUM on hardware."""
            last = (qc, m) == (NQC - 1, 1)
            ctmp = rrp.tile([P, 512], F32, name=f"ctmp{qc}_{m}", tag="ctmp", bufs=2)

            def emit_ctmp():
                for hh in range(2):
                    nc.vector.tensor_copy(
                        ctmp[hh * DK : (hh + 1) * DK, :], pvs[hh][0:DK, :]
                    )

            # ctx copies first: they free the pair's PSUM banks, gating the
            # next pair's PV.  The final pair has no successor, so there the
            # reciprocal chain (gating the tail broadcast) goes first.
            if not last:
                emit_ctmp()
            den2 = rrp.tile([1, 2, 512], F32, name=f"den{qc}_{m}", tag="den", bufs=2)
            for hh in range(2):
                nc.vector.tensor_copy(den2[:, hh, :], pvs[hh][DK : DK + 1, :])
            rcp = rrp.tile([1, 2, 512], F32, name=f"rcp{qc}_{m}", tag="rcp", bufs=2)
            nc.vector.reciprocal_approx_fast(rcp[:], den2[:])
            rcp_bf = rrp.tile([1, 2, 512], BF, name=f"rcpb{qc}_{m}", tag="rcpb", bufs=2)
            nc.vector.tensor_copy(rcp_bf[:], rcp[:])
            if last:
                emit_ctmp()
            if qc == 0 and m == 0 and "dbg_rcp" in ins:
                nc.gpsimd.dma_start(ins["dbg_rcp"][:], rcp[:])
                den = rrp.tile([1, 2, 512], F32, name="dbgden", tag="dbgden", bufs=1)
                for hh in range(2):
                    nc.vector.tensor_copy(den[:, hh, :], pvs[hh][DK : DK + 1, :])
                nc.gpsimd.dma_start(ins["dbg_den"][:], den[:])
            pending_norm.append((qc, m, ctmp, rcp_bf))

        def flush_norm():
            """PE part: one K=2 selector matmul broadcasts the pair's two
            reciprocal rows over 128 partitions; DVE multiplies into the
            packed bf16 ctx tile."""
            if not pending_norm:
                return
            qc, m, ctmp, rcp_bf = pending_norm.pop()
            rb_ps = psY.tile([P, 512], F32, name=f"rb{qc}_{m}", tag="yp")
            for hh in range(2):
                nc.tensor.matmul(
                    rb_ps[:],
                    selh[:, hh, :],
                    rcp_bf[:, hh, :],
                    start=(hh == 0),
                    stop=(hh == 1),
                )
            nc.vector.tensor_mul(ctx_sets[qc][m][:], ctmp[:], rb_ps[:])

        def scores_pair(qc, m, mid_cb=None, last_cb=None):
            """QK^T, exp, PV for head pair (2m, 2m+1), software-pipelined:
            QK(kb+1) is emitted before PV(kb) so the PE never waits on the
            exp.  mid_cb(kb) lets the schedule inject deferred work into
            the PE stream after PV(kb)."""
            nkb = 4 * qc + 4
            pvs = [
                psB.tile([P, 512], F32, name=f"ctx{qc}_{m}_{i}", tag="ctx")
                for i in range(2)
            ]
            pts = {}

            def qk(kb):
                dd = kb - 4 * qc
                qoff = max(0, dd) * P
                ps = psA.tile([P, 1024], F32, name=f"ps_a{qc}_{m}_{kb}", tag="ps")
                for hh in range(2):
                    r0 = hh * DK
                    nc.tensor.matmul(
                        ps[:, hh * 512 + qoff : (hh + 1) * 512],
                        kt_sb[r0 : r0 + DK, m, ds(kb * P, P)],
                        qt_sb[r0 : r0 + DK, m, ds(qc * 512 + qoff, 512 - qoff)],
                        start=True,
                        stop=True,
                    )
                if dd >= 0:
                    diag = ps[:].rearrange("p (h q) -> p h q", h=2)[
                        :, :, qoff : qoff + P
                    ]
                    nc.vector.tensor_add(
                        diag,
                        diag,
                        tri_sb[:]
                        .rearrange("p (a q) -> p a q", a=1)
                        .to_broadcast([P, 2, P]),
                    )
                pt = pt_pool.tile([P, 1024], BF, name=f"pt{qc}_{m}_{kb}", tag="pt")
                ps3 = ps[:].rearrange("p (h q) -> p h q", h=2)[:, :, qoff:]
                pt3 = pt[:].rearrange("p (h q) -> p h q", h=2)[:, :, qoff:]
                nc.scalar.activation(
                    pt3, ps3, AF.Exp, scale=0.125, bias=padneg_sb[:, kb : kb + 1]
                )
                pts[kb] = pt

            def pv(kb):
                dd = kb - 4 * qc
                qoff = max(0, dd) * P
                pt = pts.pop(kb)
                for hh in range(2):
                    h = 2 * m + hh
                    nc.tensor.matmul(
                        pvs[hh][:, qoff:],
                        vaug_sb[:, kb, ds(h * (DK + 1), P)],
                        pt[:, hh * 512 + qoff : (hh + 1) * 512],
                        start=(kb == 0),
                        stop=(kb == nkb - 1),
                    )

            qk(0)
            for kb in range(1, nkb):
                qk(kb)
                pv(kb - 1)
                if mid_cb is not None:
                    mid_cb(kb - 1)
            pv(nkb - 1)
            if last_cb is not None:
                last_cb(pvs)
            if mid_cb is not None:
                mid_cb(nkb - 1)
            return pvs

        yts = {}

        def outproj_step(qc, si, nch):
            def step():
                s = qc * 4 + si
                if nch == 0:
                    yts[s] = ysb.tile([P, 1024], BF, name=f"yt{s}", tag="yt")
                yt = yts[s]
                yp = psY.tile([P, 512], F32, name=f"yp{s}_{nch}", tag="yp")
                for m in range(2):
                    nc.tensor.matmul(
                        yp[:],
                        ctx_sets[qc][m][:, ts(si, P)],
                        wo_sb[:, m, ds(nch * 512, 512)],
                        start=(m == 0),
                        stop=(m == 1),
                    )
                if nch == 0:
                    nc.scalar.copy(yt[:, ts(nch, 512)], yp[:])
                else:
                    nc.vector.tensor_copy(yt[:, ts(nch, 512)], yp[:])
                # spread the tail chunk's drain over both HWDGE queues
                q_eng = nc.scalar if (qc == NQC - 1 and nch == 1) else nc.sync
                q_eng.dma_start(
                    y_r[s][:, ds(nch * 512, 512)], yt[:, ts(nch, 512)]
                )

            return step

        def outproj(qc, sis):
            for si in sis:
                for nch in range(2):
                    outproj_step(qc, si, nch)()

        # Final chunk's output projection split at the pair accumulation:
        # the pair-0 partial matmuls depend only on flush(3,0), so they run
        # as filler inside unit(3,1); f32 partials park in SBUF and only the
        # pair-1 matmul + add remain after the final flush.  Bit-identical
        # to the fused accumulation (f32 sum, bf16 on the final write).
        ym = {}

        def tail_partial0_step(si, nch):
            def step():
                ym[(si, nch)] = const.tile(
                    [P, 512], F32, name=f"ym{si}_{nch}", tag=f"ym{si}_{nch}"
                )
                yp = psY.tile([P, 512], F32, name=f"yq{si}_{nch}", tag="yp")
                nc.tensor.matmul(
                    yp[:],
                    ctx_sets[NQC - 1][0][:, ts(si, P)],
                    wo_sb[:, 0, ds(nch * 512, 512)],
                    start=True,
                    stop=True,
                )
                if nch == 0:
                    nc.scalar.copy(ym[(si, nch)][:], yp[:])
                else:
                    nc.vector.tensor_copy(ym[(si, nch)][:], yp[:])

            return step

        def tail_final(si, nch):
            s = (NQC - 1) * 4 + si
            if nch == 0:
                yts[s] = ysb.tile([P, 1024], BF, name=f"yt{s}", tag="yt")
            yt = yts[s]
            yp = psY.tile([P, 512], F32, name=f"yr{si}_{nch}", tag="yp")
            nc.tensor.matmul(
                yp[:],
                ctx_sets[NQC - 1][1][:, ts(si, P)],
                wo_sb[:, 1, ds(nch * 512, 512)],
                start=True,
                stop=True,
            )
            nc.vector.tensor_add(yt[:, ts(nch, 512)], ym[(si, nch)][:], yp[:])
            q_eng = nc.scalar if nch == 1 else nc.sync
            q_eng.dma_start(y_r[s][:, ds(nch * 512, 512)], yt[:, ts(nch, 512)])

        # ---------------- interleaved schedule ----------------
        # Emission order IS the per-engine execution order.  Attention
        # key-blocks are the clock; all other PE work (projection chunks for
        # later qcs, the previous chunk's output projection, the deferred
        # normalization broadcasts) is queued as small filler steps and one
        # is popped after every key-block, so the PE always has ~1.5us of
        # work per ~1us of exp and never idles into a HAM re-throttle.
        from collections import deque

        filler = deque()

        def unit(qc, m):
            stride = 2 if qc >= 2 else 1

            def mid(kb):
                # kb0: filler; kb1: previous pair's broadcast+normalize (its
                # DVE chain then has ~2 key-blocks of runway and the PE
                # never waits on it); kb>=2: filler by stride
                if kb == 1:
                    flush_norm()
                elif (kb == 0 or kb % stride == 1 % stride) and filler:
                    filler.popleft()()

            scores_pair(
                qc, m, mid_cb=mid, last_cb=lambda pvs: start_norm(qc, m, pvs)
            )

        def queue_outproj(qc, sis):
            for si in sis:
                for nch in range(2):
                    filler.append(outproj_step(qc, si, nch))

        # chunks 0-1 as lumps under the input-DMA shadow: q/k steps first
        # (their weights land before wv), v steps after
        s0, s1 = proj_steps(0), proj_steps(1)
        for st in s0[:4] + s1[:4] + s0[4:] + s1[4:]:
            st()
        if "dbg_vaug" in ins:
            nc.gpsimd.dma_start(ins["dbg_vaug"][:], vaug_sb[:, 0, :])

        # attention chunks qc1/qc2 are PE-rich (they carry the projection
        # steps); output projections migrate as late as their flush allows
        # so the exp-rate-limited qc3 key-blocks all have PE filler
        filler.extend(proj_steps(2))
        unit(0, 0)
        unit(0, 1)
        if "dbg_ctx" in ins:
            nc.gpsimd.dma_start(ins["dbg_ctx"][:], ctx_sets[0][0][:])
        filler.extend(proj_steps(3))
        unit(1, 0)
        unit(1, 1)
        queue_outproj(0, (0, 1, 2, 3))
        unit(2, 0)
        queue_outproj(1, (0, 1))
        unit(2, 1)
        queue_outproj(1, (2, 3))
        queue_outproj(2, (0, 1))
        unit(3, 0)
        queue_outproj(2, (2, 3))
        for si in range(4):
            for nch in range(2):
                filler.append(tail_partial0_step(si, nch))
        unit(3, 1)
        while filler:
            filler.popleft()()
        flush_norm()
        for si in range(4):
            for nch in range(2):
                tail_final(si, nch)


# ---------------- host side ----------------

def _bf16(a):
    import ml_dtypes

    return np.asarray(a, dtype=np.float32).astype(ml_dtypes.bfloat16)


def make_in_maps(x, padding_mask, Wq, bq, Wk, Wv, Wo):
    """Build the 8 per-core input dicts from full inputs."""
    x = np.asarray(x, dtype=np.float32)
    pad = np.asarray(padding_mask)
    tri = np.where(
        np.arange(P)[:, None] > np.arange(P)[None, :], np.float32(NEG), np.float32(0)
    ).astype(np.float32)
    in_maps = []
    def swz_w(w):  # [D, 256] -> [P, KT, 256] partition-contiguous
        return np.ascontiguousarray(w.reshape(KT, P, 2 * P).transpose(1, 0, 2))

    for c in range(N_CORES):
        b, g = divmod(c, 4)
        R = slice(g * 256, g * 256 + 256)
        padneg = ((pad[b] == 0) * np.float32(PADBIAS)).reshape(ST, P).T.copy()
        xt4 = x[b].T.reshape(KT, P, NQC, 512).transpose(1, 2, 0, 3)
        in_maps.append(
            {
                "xt": _bf16(np.ascontiguousarray(xt4)),
                "wq": _bf16(swz_w(np.asarray(Wq, np.float32)[R, :].T)),
                "wk": _bf16(swz_w(np.asarray(Wk, np.float32)[R, :].T)),
                "wv": _bf16(swz_w(np.asarray(Wv, np.float32)[R, :].T)),
                "wo": _bf16(
                    np.ascontiguousarray(
                        np.asarray(Wo, np.float32)[:, R].T.reshape(2, P, D).transpose(
                            1, 0, 2
                        )
                    )
                ),
                "bq": np.ascontiguousarray(
                    np.asarray(bq, np.float32)[R].reshape(2, P).T
                ),
                "padneg": np.ascontiguousarray(padneg),
                "tri": tri,
            }
        )
    return in_maps


def postprocess(partials, x, padding_mask, Wv, bv, Wo, bo):
    """Sum per-core partials, add folded bias, fix fully-masked rows."""
    x = np.asarray(x, np.float32)
    pad = np.asarray(padding_mask)
    Wv = np.asarray(Wv, np.float32)
    bv = np.asarray(bv, np.float32)
    Wo = np.asarray(Wo, np.float32)
    bo = np.asarray(bo, np.float32)
    B = x.shape[0]
    y = np.zeros((B, S, D), dtype=np.float32)
    for c in range(N_CORES):
        y[c // 4] += np.asarray(partials[c], dtype=np.float32)
    y += (Wo @ bv + bo)[None, None, :]
    # fully-masked rows (reference: uniform attention over all keys)
    for b in range(B):
        nz = np.flatnonzero(pad[b] != 0)
        q0 = int(nz[0]) if len(nz) else S
        if q0 > 0:
            ctx_u = x[b].mean(axis=0) @ Wv.T + bv
            y[b, :q0, :] = ctx_u @ Wo.T + bo
    return y


_NC_CACHE = {}


def _get_program():
    if "nc" not in _NC_CACHE:
        _NC_CACHE["nc"] = build_program()
    return _NC_CACHE["nc"]


def kernel(
    x, padding_mask, Wq, bq, Wk, bk, Wv, bv, Wo, bo
):
    from concourse.bass_utils import run_bass_kernel_spmd

    nc = _get_program()
    in_maps = make_in_maps(x, padding_mask, Wq, bq, Wk, Wv, Wo)
    res = run_bass_kernel_spmd(nc, in_maps, core_ids=list(range(N_CORES)))
    partials = [res.results[c]["y"] for c in range(N_CORES)]
    return postprocess(partials, x, padding_mask, Wv, bv, Wo, bo)


# revision 69
# speedup vs baseline: 1.0012x; 1.0012x over previous
"""Self-contained Trainium2 Bass kernel for causal multi-head attention.

Problem: B=2, S=2048, D=1024, H=16 heads (dk=64), fp32, causal + padding mask.
Sharding across 8 NeuronCores: core c -> batch c//4, head-group c%4 (4 heads).

v2 design (all-bf16 dataflow, dense PE schedule):
  - Every matmul operand is bf16 (PSUM accumulates f32): 1 cycle/column on
    the PE at any width, no fp32r narrow-N penalty, and input DMA halves.
  - Inputs stream as a handful of large DMAs split over the two HWDGE
    queues (sync: xt chunks; scalar: weights) so issue cost stays ~5us.
  - Padding mask is folded into the exp activation as a per-partition bias
    (-60000 at padded keys -> exp == 0), so V needs no zeroing and the
    softmax denominator column in V is constant 1.
  - qT/kT stored transposed [dk, S]; scores computed transposed S_T[k, q].
  - No max-subtraction in softmax (scores are O(+-10); exp cannot overflow).
  - Softmax denominator: appended ones column in V (PV matmul row 64).
  - Causal: additive -8e9 triangle on diagonal 128-blocks (pre-scale).
  - Normalization: reciprocal of the denominator read straight from PSUM,
    cast to bf16, broadcast to the pair's 128 partitions with one K=2
    selector matmul, multiplied into ctx on the PSUM->SBUF pass.  ctx for
    a head PAIR is packed into one 128-partition tile, so the output
    projection needs only 2 accumulation steps (K=128 each, no zero rows).
  - The broadcast matmul and ctx multiply for a pair are deferred into the
    NEXT pair's kb stream so the PE never waits on the DVE chain.
Fully-masked rows (all keys up to q padded) produce NaN/garbage on device
and are overwritten on host with the uniform-attention reference value.
"""

import numpy as np
from contextlib import ExitStack

import concourse.bass as bass
import concourse.bacc as bacc
import concourse.tile as tile
import concourse.mybir as mybir
from concourse.bass import ds, ts

F32 = mybir.dt.float32
BF = mybir.dt.bfloat16
AF = mybir.ActivationFunctionType

P = 128
S = 2048
D = 1024
HL = 4          # heads per core
DK = 64
KT = D // P     # 8 k-tiles over the model dim
ST = S // P     # 16 seq tiles
NQC = 4         # 512-wide query chunks
NEG = -8.0e9    # pre-scale causal mask value; *0.125 -> exp underflows to 0
PADBIAS = -60000.0  # post-scale padding bias inside exp
N_CORES = 8
N_HEAD = 16

VW = HL * (DK + 1) + DK - 1  # 323: per-head 65-wide groups, padded slice room


def build_program(num_devices=N_CORES, dbg=False):
    nc = bacc.Bacc(
        "TRN2",
        target_bir_lowering=False,
        debug=False,
        enable_asserts=True,
        num_devices=num_devices,
    )
    # all bulk inputs pre-swizzled on host into SBUF layout so every DMA
    # line is fully contiguous per partition
    ins = {
        "xt": nc.dram_tensor("xt", [P, NQC, KT, 512], BF, kind="ExternalInput").ap(),
        "wq": nc.dram_tensor("wq", [P, KT, 2 * P], BF, kind="ExternalInput").ap(),
        "wk": nc.dram_tensor("wk", [P, KT, 2 * P], BF, kind="ExternalInput").ap(),
        "wv": nc.dram_tensor("wv", [P, KT, 2 * P], BF, kind="ExternalInput").ap(),
        "wo": nc.dram_tensor("wo", [P, 2, D], BF, kind="ExternalInput").ap(),
        "bq": nc.dram_tensor("bq", [P, 2], F32, kind="ExternalInput").ap(),
        "padneg": nc.dram_tensor("padneg", [P, ST], F32, kind="ExternalInput").ap(),
        "tri": nc.dram_tensor("tri", [P, P], F32, kind="ExternalInput").ap(),
    }
    y = nc.dram_tensor("y", [S, D], BF, kind="ExternalOutput").ap()
    if dbg:
        ins["dbg_vaug"] = nc.dram_tensor(
            "dbg_vaug", [P, VW], BF, kind="ExternalOutput"
        ).ap()
        ins["dbg_rcp"] = nc.dram_tensor(
            "dbg_rcp", [1, 2, 512], F32, kind="ExternalOutput"
        ).ap()
        ins["dbg_ctx"] = nc.dram_tensor(
            "dbg_ctx", [P, 512], BF, kind="ExternalOutput"
        ).ap()
        ins["dbg_den"] = nc.dram_tensor(
            "dbg_den", [1, 2, 512], F32, kind="ExternalOutput"
        ).ap()

    with tile.TileContext(nc) as tc:
        _body(tc, y, ins)

    nc.compile()
    return nc


def _body(tc, y, ins):
    nc = tc.nc

    with ExitStack() as ctx:
        const = ctx.enter_context(tc.tile_pool(name="const", bufs=1))
        pt_pool = ctx.enter_context(tc.tile_pool(name="pt", bufs=3))
        rrp = ctx.enter_context(tc.tile_pool(name="rr", bufs=2))
        ysb = ctx.enter_context(tc.tile_pool(name="ysb", bufs=2))
        psA = ctx.enter_context(tc.tile_pool(name="psA", bufs=2, space="PSUM"))
        psB = ctx.enter_context(tc.tile_pool(name="psB", bufs=2, space="PSUM"))
        psY = ctx.enter_context(tc.tile_pool(name="psY", bufs=2, space="PSUM"))

        # ---------------- input DMAs ----------------
        # sync HWDGE queue: the four 1MB xt chunks (needed in order).
        # scalar HWDGE queue: weights + small constants.  All transfers are
        # contiguous per partition (host pre-swizzled).
        xt_sb = const.tile([P, NQC, KT, 512], BF)
        wq_sb = const.tile([P, KT, 2 * P], BF)
        wk_sb = const.tile([P, KT, 2 * P], BF)
        wv_sb = const.tile([P, KT, 2 * P], BF)
        nc.scalar.dma_start(wq_sb[:], ins["wq"])
        nc.sync.dma_start(xt_sb[:, 0], ins["xt"][:, 0])
        nc.scalar.dma_start(wk_sb[:], ins["wk"])
        nc.scalar.dma_start(wv_sb[:], ins["wv"])
        for n in range(1, 4):
            nc.sync.dma_start(xt_sb[:, n], ins["xt"][:, n])
        # bq is needed by the first q-projection copy, so it goes HWDGE
        bq_sb = const.tile([P, 2], F32)
        nc.scalar.dma_start(bq_sb[:], ins["bq"])
        padneg_sb = const.tile([P, ST], F32)
        nc.gpsimd.dma_start(padneg_sb[:], ins["padneg"])
        tri_sb = const.tile([P, P], F32)
        nc.gpsimd.dma_start(tri_sb[:], ins["tri"])
        # wo packed per head pair: partition r, pair m -> Wo column g*256+m*128+r
        wo_sb = const.tile([P, 2, D], BF)
        nc.gpsimd.dma_start(wo_sb[:], ins["wo"])

        ones_sb = const.tile([1, 512], BF)
        nc.vector.memset(ones_sb[:], 1.0)
        # selectors for the denominator broadcast (partition-0 rows; engine
        # ops may not start at partition 1): selh[0] targets partitions
        # 0-63, selh[1] targets 64-127 via two K=1 accumulating matmuls
        selh = const.tile([1, 2, P], BF)
        nc.vector.memset(selh[:], 0.0)
        nc.vector.memset(selh[:, 0, 0:DK], 1.0)
        nc.vector.memset(selh[:, 1, DK:P], 1.0)

        qt_sb = const.tile([P, 2, S], BF)
        kt_sb = const.tile([P, 2, S], BF)
        # per head: 64 value cols + 1 all-ones denominator col; padded so a
        # 128-wide stationary slice starting at h*65 stays in bounds (the
        # extra columns produce junk output rows 65-127, never read)
        vaug_sb = const.tile([P, ST, VW], BF)
        nc.vector.memset(vaug_sb[:, :, HL * (DK + 1) : VW], 0.0)
        den_cols = vaug_sb[:, :, 0 : HL * (DK + 1)].rearrange(
            "p s (h c) -> p s h c", c=DK + 1
        )[:, :, :, DK : DK + 1]
        nc.vector.memset(den_cols, 1.0)

        # normalized per-PAIR context [h0 dims 0-63 | h1 dims 64-127];
        # one set per query chunk (no reuse), so output-projection filler for
        # chunk qc can run arbitrarily late without WAR pressure
        ctx_sets = [
            [
                const.tile([P, 512], BF, name=f"ctxsb{st}_{m}", tag=f"ctxsb{st}_{m}")
                for m in range(2)
            ]
            for st in range(NQC)
        ]

        # PE warmup while the input DMAs stream (HAM un-throttle needs
        # ~3.4us of sustained matmul activity; these are dep-free)
        warm_ps = psY.tile([P, 512], F32, name="warm", tag="yp")
        for i in range(20):
            nc.tensor.matmul(
                warm_ps[:], ones_sb[:, 0:P], ones_sb[:], start=True, stop=True
            )

        # ---------------- projections for one 512-token chunk ----------------
        # Emitted as self-contained "steps" (~1.7-4us of PE work each) so the
        # schedule can sprinkle them between attention key-blocks.
        def proj_qk_step(n, tgt, w_sb, bias, m):
            def step():
                ps = psA.tile([P, 1024], F32, name=f"ps_p{n}{m}", tag="ps")
                for k in range(KT):
                    nc.tensor.matmul(
                        ps[:, 0:512],
                        w_sb[:, k, ts(m, P)],
                        xt_sb[:, n, k, :],
                        start=(k == 0),
                        stop=(k == KT - 1),
                    )
                out_ap = tgt[:, m, ds(n * 512, 512)]
                if bias is not None:
                    nc.vector.tensor_scalar_add(
                        out_ap, ps[:, 0:512], bias[:, m : m + 1]
                    )
                else:
                    nc.vector.tensor_copy(out_ap, ps[:, 0:512])

            return step

        def proj_v_step(n, si):
            def step():
                s = n * 4 + si
                ps = psA.tile([P, 1024], F32, name=f"ps_v{s}", tag="ps")
                for k in range(KT):
                    nc.tensor.matmul(
                        ps[:, 0:256],
                        xt_sb[:, n, k, ts(si, P)],
                        wv_sb[:, k, :],
                        start=(k == 0),
                        stop=(k == KT - 1),
                    )
                vdst = vaug_sb[:, s, 0 : HL * (DK + 1)].rearrange(
                    "p (h c) -> p h c", c=DK + 1
                )[:, :, 0:DK]
                vsrc = ps[:, 0:256].rearrange("p (h c) -> p h c", c=DK)
                nc.vector.tensor_copy(vdst, vsrc)

            return step

        def proj_steps(n):
            out = []
            for m in range(2):
                out.append(proj_qk_step(n, qt_sb, wq_sb, bq_sb, m))
                out.append(proj_qk_step(n, kt_sb, wk_sb, None, m))
            for si in range(4):
                out.append(proj_v_step(n, si))
            return out

        def proj_chunk(n):
            for st in proj_steps(n):
                st()

        # ---------------- attention for one 512-query chunk ----------------
        y_r = y.rearrange("(t p) n -> t p n", p=P)

        # deferred normalization state: [(qc, m, pvs, ctxtmp, rcp_bf)]
        pending_norm = []

        def start_norm(qc, m, pvs):
            """ctx copies to SBUF + reciprocal of the denominator rows.
            The ctx copies free the pair's PSUM banks (which gate the next
            pair's PV), so they come first — except for the final pair,
            where the reciprocal chain (gating the tail broadcast matmul)
            gets priority.  The denominator rows bounce through SBUF: DVE
            reciprocal_approx_fast reads garbage from PSUM on hardware."""
            last = (qc, m) == (NQC - 1, 1)
            ctmp = rrp.tile([P, 512], F32, name=f"ctmp{qc}_{m}", tag="ctmp", bufs=2)

            def emit_ctmp():
                for hh in range(2):
                    nc.vector.tensor_copy(
                        ctmp[hh * DK : (hh + 1) * DK, :], pvs[hh][0:DK, :]
                    )

            # ctx copies first: they free the pair's PSUM banks, gating the
            # next pair's PV.  The final pair has no successor, so there the
            # reciprocal chain (gating the tail broadcast) goes first.
            if not last:
                emit_ctmp()
            den2 = rrp.tile([1, 2, 512], F32, name=f"den{qc}_{m}", tag="den", bufs=2)
            for hh in range(2):
                nc.vector.tensor_copy(den2[:, hh, :], pvs[hh][DK : DK + 1, :])
            rcp = rrp.tile([1, 2, 512], F32, name=f"rcp{qc}_{m}", tag="rcp", bufs=2)
            nc.vector.reciprocal_approx_fast(rcp[:], den2[:])
            rcp_bf = rrp.tile([1, 2, 512], BF, name=f"rcpb{qc}_{m}", tag="rcpb", bufs=2)
            nc.vector.tensor_copy(rcp_bf[:], rcp[:])
            if last:
                emit_ctmp()
            if qc == 0 and m == 0 and "dbg_rcp" in ins:
                nc.gpsimd.dma_start(ins["dbg_rcp"][:], rcp[:])
                den = rrp.tile([1, 2, 512], F32, name="dbgden", tag="dbgden", bufs=1)
                for hh in range(2):
                    nc.vector.tensor_copy(den[:, hh, :], pvs[hh][DK : DK + 1, :])
                nc.gpsimd.dma_start(ins["dbg_den"][:], den[:])
            pending_norm.append((qc, m, ctmp, rcp_bf))

        def flush_norm():
            """PE part: one K=2 selector matmul broadcasts the pair's two
            reciprocal rows over 128 partitions; DVE multiplies into the
            packed bf16 ctx tile."""
            if not pending_norm:
                return
            qc, m, ctmp, rcp_bf = pending_norm.pop()
            rb_ps = psY.tile([P, 512], F32, name=f"rb{qc}_{m}", tag="yp")
            for hh in range(2):
                nc.tensor.matmul(
                    rb_ps[:],
                    selh[:, hh, :],
                    rcp_bf[:, hh, :],
                    start=(hh == 0),
                    stop=(hh == 1),
                )
            nc.vector.tensor_mul(ctx_sets[qc][m][:], ctmp[:], rb_ps[:])

        def scores_pair(qc, m, mid_cb=None, last_cb=None):
            """QK^T, exp, PV for head pair (2m, 2m+1), software-pipelined:
            QK(kb+1) is emitted before PV(kb) so the PE never waits on the
            exp.  mid_cb(kb) lets the schedule inject deferred work into
            the PE stream after PV(kb)."""
            nkb = 4 * qc + 4
            pvs = [
                psB.tile([P, 512], F32, name=f"ctx{qc}_{m}_{i}", tag="ctx")
                for i in range(2)
            ]
            pts = {}

            def qk(kb):
                dd = kb - 4 * qc
                qoff = max(0, dd) * P
                ps = psA.tile([P, 1024], F32, name=f"ps_a{qc}_{m}_{kb}", tag="ps")
                for hh in range(2):
                    r0 = hh * DK
                    nc.tensor.matmul(
                        ps[:, hh * 512 + qoff : (hh + 1) * 512],
                        kt_sb[r0 : r0 + DK, m, ds(kb * P, P)],
                        qt_sb[r0 : r0 + DK, m, ds(qc * 512 + qoff, 512 - qoff)],
                        start=True,
                        stop=True,
                    )
                if dd >= 0:
                    diag = ps[:].rearrange("p (h q) -> p h q", h=2)[
                        :, :, qoff : qoff + P
                    ]
                    nc.vector.tensor_add(
                        diag,
                        diag,
                        tri_sb[:]
                        .rearrange("p (a q) -> p a q", a=1)
                        .to_broadcast([P, 2, P]),
                    )
                pt = pt_pool.tile([P, 1024], BF, name=f"pt{qc}_{m}_{kb}", tag="pt")
                ps3 = ps[:].rearrange("p (h q) -> p h q", h=2)[:, :, qoff:]
                pt3 = pt[:].rearrange("p (h q) -> p h q", h=2)[:, :, qoff:]
                nc.scalar.activation(
                    pt3, ps3, AF.Exp, scale=0.125, bias=padneg_sb[:, kb : kb + 1]
                )
                pts[kb] = pt

            def pv(kb):
                dd = kb - 4 * qc
                qoff = max(0, dd) * P
                pt = pts.pop(kb)
                for hh in range(2):
                    h = 2 * m + hh
                    nc.tensor.matmul(
                        pvs[hh][:, qoff:],
                        vaug_sb[:, kb, ds(h * (DK + 1), P)],
                        pt[:, hh * 512 + qoff : (hh + 1) * 512],
                        start=(kb == 0),
                        stop=(kb == nkb - 1),
                    )

            qk(0)
            for kb in range(1, nkb):
                qk(kb)
                pv(kb - 1)
                if mid_cb is not None:
                    mid_cb(kb - 1)
            pv(nkb - 1)
            if last_cb is not None:
                last_cb(pvs)
            if mid_cb is not None:
                mid_cb(nkb - 1)
            return pvs

        yts = {}

        def outproj_step(qc, si, nch):
            def step():
                s = qc * 4 + si
                if nch == 0:
                    yts[s] = ysb.tile([P, 1024], BF, name=f"yt{s}", tag="yt")
                yt = yts[s]
                yp = psY.tile([P, 512], F32, name=f"yp{s}_{nch}", tag="yp")
                for m in range(2):
                    nc.tensor.matmul(
                        yp[:],
                        ctx_sets[qc][m][:, ts(si, P)],
                        wo_sb[:, m, ds(nch * 512, 512)],
                        start=(m == 0),
                        stop=(m == 1),
                    )
                if nch == 0:
                    nc.scalar.copy(yt[:, ts(nch, 512)], yp[:])
                else:
                    nc.vector.tensor_copy(yt[:, ts(nch, 512)], yp[:])
                # spread the tail chunk's drain over both HWDGE queues
                q_eng = nc.scalar if (qc == NQC - 1 and nch == 1) else nc.sync
                q_eng.dma_start(
                    y_r[s][:, ds(nch * 512, 512)], yt[:, ts(nch, 512)]
                )

            return step

        def outproj(qc, sis):
            for si in sis:
                for nch in range(2):
                    outproj_step(qc, si, nch)()

        # ---------------- interleaved schedule ----------------
        # Emission order IS the per-engine execution order.  Attention
        # key-blocks are the clock; all other PE work (projection chunks for
        # later qcs, the previous chunk's output projection, the deferred
        # normalization broadcasts) is queued as small filler steps and one
        # is popped after every key-block, so the PE always has ~1.5us of
        # work per ~1us of exp and never idles into a HAM re-throttle.
        from collections import deque

        filler = deque()

        def unit(qc, m):
            stride = 4 if (qc, m) == (3, 1) else (2 if qc >= 2 else 1)

            def mid(kb):
                # kb0: filler; kb1: previous pair's broadcast+normalize (its
                # DVE chain then has ~2 key-blocks of runway and the PE
                # never waits on it); kb>=2: filler by stride
                if kb == 1:
                    flush_norm()
                elif (kb == 0 or kb % stride == 1 % stride) and filler:
                    filler.popleft()()

            scores_pair(
                qc, m, mid_cb=mid, last_cb=lambda pvs: start_norm(qc, m, pvs)
            )

        def queue_outproj(qc, sis):
            for si in sis:
                for nch in range(2):
                    filler.append(outproj_step(qc, si, nch))

        # chunks 0-1 as lumps under the input-DMA shadow: q/k steps first
        # (their weights land before wv), v steps after
        s0, s1 = proj_steps(0), proj_steps(1)
        for st in s0[:4] + s1[:4] + s0[4:] + s1[4:]:
            st()
        if "dbg_vaug" in ins:
            nc.gpsimd.dma_start(ins["dbg_vaug"][:], vaug_sb[:, 0, :])

        # attention chunks qc1/qc2 are PE-rich (they carry the projection
        # steps); output projections migrate as late as their flush allows
        # so the exp-rate-limited qc3 key-blocks all have PE filler
        filler.extend(proj_steps(2))
        unit(0, 0)
        unit(0, 1)
        if "dbg_ctx" in ins:
            nc.gpsimd.dma_start(ins["dbg_ctx"][:], ctx_sets[0][0][:])
        filler.extend(proj_steps(3))
        unit(1, 0)
        unit(1, 1)
        queue_outproj(0, (0, 1, 2, 3))
        unit(2, 0)
        queue_outproj(1, (0, 1))
        unit(2, 1)
        queue_outproj(1, (2, 3))
        queue_outproj(2, (0, 1))
        unit(3, 0)
        queue_outproj(2, (2, 3))
        unit(3, 1)
        while filler:
            filler.popleft()()
        flush_norm()
        outproj(NQC - 1, (0, 1, 2, 3))


# ---------------- host side ----------------

def _bf16(a):
    import ml_dtypes

    return np.asarray(a, dtype=np.float32).astype(ml_dtypes.bfloat16)


def make_in_maps(x, padding_mask, Wq, bq, Wk, Wv, Wo):
    """Build the 8 per-core input dicts from full inputs."""
    x = np.asarray(x, dtype=np.float32)
    pad = np.asarray(padding_mask)
    tri = np.where(
        np.arange(P)[:, None] > np.arange(P)[None, :], np.float32(NEG), np.float32(0)
    ).astype(np.float32)
    in_maps = []
    def swz_w(w):  # [D, 256] -> [P, KT, 256] partition-contiguous
        return np.ascontiguousarray(w.reshape(KT, P, 2 * P).transpose(1, 0, 2))

    for c in range(N_CORES):
        b, g = divmod(c, 4)
        R = slice(g * 256, g * 256 + 256)
        padneg = ((pad[b] == 0) * np.float32(PADBIAS)).reshape(ST, P).T.copy()
        xt4 = x[b].T.reshape(KT, P, NQC, 512).transpose(1, 2, 0, 3)
        in_maps.append(
            {
                "xt": _bf16(np.ascontiguousarray(xt4)),
                "wq": _bf16(swz_w(np.asarray(Wq, np.float32)[R, :].T)),
                "wk": _bf16(swz_w(np.asarray(Wk, np.float32)[R, :].T)),
                "wv": _bf16(swz_w(np.asarray(Wv, np.float32)[R, :].T)),
                "wo": _bf16(
                    np.ascontiguousarray(
                        np.asarray(Wo, np.float32)[:, R].T.reshape(2, P, D).transpose(
                            1, 0, 2
                        )
                    )
                ),
                "bq": np.ascontiguousarray(
                    np.asarray(bq, np.float32)[R].reshape(2, P).T
                ),
                "padneg": np.ascontiguousarray(padneg),
                "tri": tri,
            }
        )
    return in_maps


def postprocess(partials, x, padding_mask, Wv, bv, Wo, bo):
    """Sum per-core partials, add folded bias, fix fully-masked rows."""
    x = np.asarray(x, np.float32)
    pad = np.asarray(padding_mask)
    Wv = np.asarray(Wv, np.float32)
    bv = np.asarray(bv, np.float32)
    Wo = np.asarray(Wo, np.float32)
    bo = np.asarray(bo, np.float32)
    B = x.shape[0]
    y = np.zeros((B, S, D), dtype=np.float32)
    for c in range(N_CORES):
        y[c // 4] += np.asarray(partials[c], dtype=np.float32)
    y += (Wo @ bv + bo)[None, None, :]
    # fully-masked rows (reference: uniform attention over all keys)
    for b in range(B):
        nz = np.flatnonzero(pad[b] != 0)
        q0 = int(nz[0]) if len(nz) else S
        if q0 > 0:
            ctx_u = x[b].mean(axis=0) @ Wv.T + bv
            y[b, :q0, :] = ctx_u @ Wo.T + bo
    return y


_NC_CACHE = {}


def _get_program():
    if "nc" not in _NC_CACHE:
        _NC_CACHE["nc"] = build_program()
    return _NC_CACHE["nc"]


def kernel(
    x, padding_mask, Wq, bq, Wk, bk, Wv, bv, Wo, bo
):
    from concourse.bass_utils import run_bass_kernel_spmd

    nc = _get_program()
    in_maps = make_in_maps(x, padding_mask, Wq, bq, Wk, Wv, Wo)
    res = run_bass_kernel_spmd(nc, in_maps, core_ids=list(range(N_CORES)))
    partials = [res.results[c]["y"] for c in range(N_CORES)]
    return postprocess(partials, x, padding_mask, Wv, bv, Wo, bo)


# revision 70
# speedup vs baseline: 1.1676x; 1.1663x over previous
"""Self-contained Trainium2 Bass kernel for causal multi-head attention.

Problem: B=2, S=2048, D=1024, H=16 heads (dk=64), fp32, causal + padding mask.
Sharding across 8 NeuronCores: core c -> batch c//4, head-group c%4 (4 heads).

v2 design (all-bf16 dataflow, dense PE schedule):
  - Every matmul operand is bf16 (PSUM accumulates f32): 1 cycle/column on
    the PE at any width, no fp32r narrow-N penalty, and input DMA halves.
  - Inputs stream as a handful of large DMAs split over the two HWDGE
    queues (sync: xt chunks; scalar: weights) so issue cost stays ~5us.
  - Padding mask is folded into the exp activation as a per-partition bias
    (-60000 at padded keys -> exp == 0), so V needs no zeroing and the
    softmax denominator column in V is constant 1.
  - qT/kT stored transposed [dk, S]; scores computed transposed S_T[k, q].
  - No max-subtraction in softmax (scores are O(+-10); exp cannot overflow).
  - Softmax denominator: appended ones column in V (PV matmul row 64).
  - Causal: additive -8e9 triangle on diagonal 128-blocks (pre-scale).
  - Normalization: reciprocal of the denominator read straight from PSUM,
    cast to bf16, broadcast to the pair's 128 partitions with one K=2
    selector matmul, multiplied into ctx on the PSUM->SBUF pass.  ctx for
    a head PAIR is packed into one 128-partition tile, so the output
    projection needs only 2 accumulation steps (K=128 each, no zero rows).
  - The broadcast matmul and ctx multiply for a pair are deferred into the
    NEXT pair's kb stream so the PE never waits on the DVE chain.
Fully-masked rows (all keys up to q padded) produce NaN/garbage on device
and are overwritten on host with the uniform-attention reference value.
"""

import numpy as np
from contextlib import ExitStack

import concourse.bass as bass
import concourse.bacc as bacc
import concourse.tile as tile
import concourse.mybir as mybir
from concourse.bass import ds, ts

F32 = mybir.dt.float32
BF = mybir.dt.bfloat16
AF = mybir.ActivationFunctionType

P = 128
S = 2048
D = 1024
HL = 4          # heads per core
DK = 64
KT = D // P     # 8 k-tiles over the model dim
ST = S // P     # 16 seq tiles
NQC = 4         # 512-wide query chunks
NEG = -8.0e9    # pre-scale causal mask value; *0.125 -> exp underflows to 0
PADBIAS = -60000.0  # post-scale padding bias inside exp
N_CORES = 8
N_HEAD = 16

VW = HL * (DK + 1) + DK - 1  # 323: per-head 65-wide groups, padded slice room


def build_program(num_devices=N_CORES, dbg=False):
    nc = bacc.Bacc(
        "TRN2",
        target_bir_lowering=False,
        debug=False,
        enable_asserts=True,
        num_devices=num_devices,
    )
    # all bulk inputs pre-swizzled on host into SBUF layout so every DMA
    # line is fully contiguous per partition
    ins = {
        "xt": nc.dram_tensor("xt", [P, NQC, KT, 512], BF, kind="ExternalInput").ap(),
        "wq": nc.dram_tensor("wq", [P, KT, 2 * P], BF, kind="ExternalInput").ap(),
        "wk": nc.dram_tensor("wk", [P, KT, 2 * P], BF, kind="ExternalInput").ap(),
        "wv": nc.dram_tensor("wv", [P, KT, 2 * P], BF, kind="ExternalInput").ap(),
        "wo": nc.dram_tensor("wo", [P, 2, D], BF, kind="ExternalInput").ap(),
        "bq": nc.dram_tensor("bq", [P, 2], F32, kind="ExternalInput").ap(),
        "padneg": nc.dram_tensor("padneg", [P, ST], F32, kind="ExternalInput").ap(),
        "tri": nc.dram_tensor("tri", [P, P], F32, kind="ExternalInput").ap(),
    }
    y = nc.dram_tensor("y", [S, D], BF, kind="ExternalOutput").ap()
    if dbg:
        ins["dbg_vaug"] = nc.dram_tensor(
            "dbg_vaug", [P, VW], BF, kind="ExternalOutput"
        ).ap()
        ins["dbg_rcp"] = nc.dram_tensor(
            "dbg_rcp", [1, 2, 512], F32, kind="ExternalOutput"
        ).ap()
        ins["dbg_ctx"] = nc.dram_tensor(
            "dbg_ctx", [P, 512], BF, kind="ExternalOutput"
        ).ap()
        ins["dbg_den"] = nc.dram_tensor(
            "dbg_den", [1, 2, 512], F32, kind="ExternalOutput"
        ).ap()

    with tile.TileContext(nc) as tc:
        _body(tc, y, ins)

    nc.compile()
    return nc


def _body(tc, y, ins):
    nc = tc.nc

    with ExitStack() as ctx:
        const = ctx.enter_context(tc.tile_pool(name="const", bufs=1))
        pt_pool = ctx.enter_context(tc.tile_pool(name="pt", bufs=3))
        rrp = ctx.enter_context(tc.tile_pool(name="rr", bufs=2))
        ysb = ctx.enter_context(tc.tile_pool(name="ysb", bufs=2))
        psA = ctx.enter_context(tc.tile_pool(name="psA", bufs=2, space="PSUM"))
        psB = ctx.enter_context(tc.tile_pool(name="psB", bufs=2, space="PSUM"))
        psY = ctx.enter_context(tc.tile_pool(name="psY", bufs=2, space="PSUM"))

        # ---------------- input DMAs ----------------
        # sync HWDGE queue: the four 1MB xt chunks (needed in order).
        # scalar HWDGE queue: weights + small constants.  All transfers are
        # contiguous per partition (host pre-swizzled).
        xt_sb = const.tile([P, NQC, KT, 512], BF)
        wq_sb = const.tile([P, KT, 2 * P], BF)
        wk_sb = const.tile([P, KT, 2 * P], BF)
        wv_sb = const.tile([P, KT, 2 * P], BF)
        nc.scalar.dma_start(wq_sb[:], ins["wq"])
        nc.sync.dma_start(xt_sb[:, 0], ins["xt"][:, 0])
        nc.scalar.dma_start(wk_sb[:], ins["wk"])
        nc.scalar.dma_start(wv_sb[:], ins["wv"])
        for n in range(1, 4):
            nc.sync.dma_start(xt_sb[:, n], ins["xt"][:, n])
        # bq is needed by the first q-projection copy, so it goes HWDGE
        bq_sb = const.tile([P, 2], F32)
        nc.scalar.dma_start(bq_sb[:], ins["bq"])
        padneg_sb = const.tile([P, ST], F32)
        nc.gpsimd.dma_start(padneg_sb[:], ins["padneg"])
        tri_sb = const.tile([P, P], F32)
        nc.gpsimd.dma_start(tri_sb[:], ins["tri"])
        # wo packed per head pair: partition r, pair m -> Wo column g*256+m*128+r
        wo_sb = const.tile([P, 2, D], BF)
        nc.gpsimd.dma_start(wo_sb[:], ins["wo"])

        ones_sb = const.tile([1, 512], BF)
        nc.vector.memset(ones_sb[:], 1.0)
        # selectors for the denominator broadcast (partition-0 rows; engine
        # ops may not start at partition 1): selh[0] targets partitions
        # 0-63, selh[1] targets 64-127 via two K=1 accumulating matmuls
        selh = const.tile([1, 2, P], BF)
        nc.vector.memset(selh[:], 0.0)
        nc.vector.memset(selh[:, 0, 0:DK], 1.0)
        nc.vector.memset(selh[:, 1, DK:P], 1.0)

        qt_sb = const.tile([P, 2, S], BF)
        kt_sb = const.tile([P, 2, S], BF)
        # per head: 64 value cols + 1 all-ones denominator col; padded so a
        # 128-wide stationary slice starting at h*65 stays in bounds (the
        # extra columns produce junk output rows 65-127, never read)
        vaug_sb = const.tile([P, ST, VW], BF)
        nc.vector.memset(vaug_sb[:, :, HL * (DK + 1) : VW], 0.0)
        den_cols = vaug_sb[:, :, 0 : HL * (DK + 1)].rearrange(
            "p s (h c) -> p s h c", c=DK + 1
        )[:, :, :, DK : DK + 1]
        nc.vector.memset(den_cols, 1.0)

        # normalized per-PAIR context [h0 dims 0-63 | h1 dims 64-127];
        # one set per query chunk (no reuse), so output-projection filler for
        # chunk qc can run arbitrarily late without WAR pressure
        ctx_sets = [
            [
                const.tile([P, 512], BF, name=f"ctxsb{st}_{m}", tag=f"ctxsb{st}_{m}")
                for m in range(2)
            ]
            for st in range(NQC)
        ]

        # PE warmup while the input DMAs stream (HAM un-throttle needs
        # ~3.4us of sustained matmul activity; these are dep-free)
        warm_ps = psY.tile([P, 512], F32, name="warm", tag="yp")
        for i in range(20):
            nc.tensor.matmul(
                warm_ps[:], ones_sb[:, 0:P], ones_sb[:], start=True, stop=True
            )

        # ---------------- projections for one 512-token chunk ----------------
        # Emitted as self-contained "steps" (~1.7-4us of PE work each) so the
        # schedule can sprinkle them between attention key-blocks.
        def proj_qk_step(n, tgt, w_sb, bias, m):
            def step():
                ps = psA.tile([P, 1024], F32, name=f"ps_p{n}{m}", tag="ps")
                for k in range(KT):
                    nc.tensor.matmul(
                        ps[:, 0:512],
                        w_sb[:, k, ts(m, P)],
                        xt_sb[:, n, k, :],
                        start=(k == 0),
                        stop=(k == KT - 1),
                    )
                out_ap = tgt[:, m, ds(n * 512, 512)]
                if bias is not None:
                    nc.vector.tensor_scalar_add(
                        out_ap, ps[:, 0:512], bias[:, m : m + 1]
                    )
                else:
                    nc.vector.tensor_copy(out_ap, ps[:, 0:512])

            return step

        def proj_v_step(n, si):
            def step():
                s = n * 4 + si
                ps = psA.tile([P, 1024], F32, name=f"ps_v{s}", tag="ps")
                for k in range(KT):
                    nc.tensor.matmul(
                        ps[:, 0:256],
                        xt_sb[:, n, k, ts(si, P)],
                        wv_sb[:, k, :],
                        start=(k == 0),
                        stop=(k == KT - 1),
                    )
                vdst = vaug_sb[:, s, 0 : HL * (DK + 1)].rearrange(
                    "p (h c) -> p h c", c=DK + 1
                )[:, :, 0:DK]
                vsrc = ps[:, 0:256].rearrange("p (h c) -> p h c", c=DK)
                nc.vector.tensor_copy(vdst, vsrc)

            return step

        def proj_steps(n):
            out = []
            for m in range(2):
                out.append(proj_qk_step(n, qt_sb, wq_sb, bq_sb, m))
                out.append(proj_qk_step(n, kt_sb, wk_sb, None, m))
            for si in range(4):
                out.append(proj_v_step(n, si))
            return out

        def proj_chunk(n):
            for st in proj_steps(n):
                st()

        # ---------------- attention for one 512-query chunk ----------------
        y_r = y.rearrange("(t p) n -> t p n", p=P)

        # deferred normalization state: [(qc, m, pvs, ctxtmp, rcp_bf)]
        pending_norm = []

        def start_norm(qc, m, pvs):
            """ctx copies to SBUF + reciprocal of the denominator rows.
            The ctx copies free the pair's PSUM banks (which gate the next
            pair's PV), so they come first — except for the final pair,
            where the reciprocal chain (gating the tail broadcast matmul)
            gets priority.  The denominator rows bounce through SBUF: DVE
            reciprocal_approx_fast reads garbage from PSUM on hardware."""
            last = (qc, m) == (NQC - 1, 1)
            ctmp = rrp.tile([P, 512], F32, name=f"ctmp{qc}_{m}", tag="ctmp", bufs=2)

            def emit_ctmp():
                for hh in range(2):
                    nc.vector.tensor_copy(
                        ctmp[hh * DK : (hh + 1) * DK, :], pvs[hh][0:DK, :]
                    )

            # ctx copies first: they free the pair's PSUM banks, gating the
            # next pair's PV.  The final pair has no successor, so there the
            # reciprocal chain (gating the tail broadcast) goes first.
            if not last:
                emit_ctmp()
            den2 = rrp.tile([1, 2, 512], F32, name=f"den{qc}_{m}", tag="den", bufs=2)
            for hh in range(2):
                nc.vector.tensor_copy(den2[:, hh, :], pvs[hh][DK : DK + 1, :])
            rcp = rrp.tile([1, 2, 512], F32, name=f"rcp{qc}_{m}", tag="rcp", bufs=2)
            nc.vector.reciprocal_approx_fast(rcp[:], den2[:])
            rcp_bf = rrp.tile([1, 2, 512], BF, name=f"rcpb{qc}_{m}", tag="rcpb", bufs=2)
            nc.vector.tensor_copy(rcp_bf[:], rcp[:])
            if last:
                emit_ctmp()
            if qc == 0 and m == 0 and "dbg_rcp" in ins:
                nc.gpsimd.dma_start(ins["dbg_rcp"][:], rcp[:])
                den = rrp.tile([1, 2, 512], F32, name="dbgden", tag="dbgden", bufs=1)
                for hh in range(2):
                    nc.vector.tensor_copy(den[:, hh, :], pvs[hh][DK : DK + 1, :])
                nc.gpsimd.dma_start(ins["dbg_den"][:], den[:])
            pending_norm.append((qc, m, ctmp, rcp_bf))

        def flush_norm():
            """PE part: one K=2 selector matmul broadcasts the pair's two
            reciprocal rows over 128 partitions; DVE multiplies into the
            packed bf16 ctx tile."""
            if not pending_norm:
                return
            qc, m, ctmp, rcp_bf = pending_norm.pop()
            rb_ps = psY.tile([P, 512], F32, name=f"rb{qc}_{m}", tag="yp")
            for hh in range(2):
                nc.tensor.matmul(
                    rb_ps[:],
                    selh[:, hh, :],
                    rcp_bf[:, hh, :],
                    start=(hh == 0),
                    stop=(hh == 1),
                )
            nc.vector.tensor_mul(ctx_sets[qc][m][:], ctmp[:], rb_ps[:])

        def scores_pair(qc, m, mid_cb=None, last_cb=None):
            """QK^T, exp, PV for head pair (2m, 2m+1), software-pipelined:
            QK(kb+1) is emitted before PV(kb) so the PE never waits on the
            exp.  mid_cb(kb) lets the schedule inject deferred work into
            the PE stream after PV(kb)."""
            nkb = 4 * qc + 4
            pvs = [
                psB.tile([P, 512], F32, name=f"ctx{qc}_{m}_{i}", tag="ctx")
                for i in range(2)
            ]
            pts = {}

            def qk(kb):
                dd = kb - 4 * qc
                qoff = max(0, dd) * P
                ps = psA.tile([P, 1024], F32, name=f"ps_a{qc}_{m}_{kb}", tag="ps")
                for hh in range(2):
                    r0 = hh * DK
                    nc.tensor.matmul(
                        ps[:, hh * 512 + qoff : (hh + 1) * 512],
                        kt_sb[r0 : r0 + DK, m, ds(kb * P, P)],
                        qt_sb[r0 : r0 + DK, m, ds(qc * 512 + qoff, 512 - qoff)],
                        start=True,
                        stop=True,
                    )
                if dd >= 0:
                    diag = ps[:].rearrange("p (h q) -> p h q", h=2)[
                        :, :, qoff : qoff + P
                    ]
                    nc.vector.tensor_add(
                        diag,
                        diag,
                        tri_sb[:]
                        .rearrange("p (a q) -> p a q", a=1)
                        .to_broadcast([P, 2, P]),
                    )
                pt = pt_pool.tile([P, 1024], BF, name=f"pt{qc}_{m}_{kb}", tag="pt")
                ps3 = ps[:].rearrange("p (h q) -> p h q", h=2)[:, :, qoff:]
                pt3 = pt[:].rearrange("p (h q) -> p h q", h=2)[:, :, qoff:]
                nc.scalar.activation(
                    pt3, ps3, AF.Exp, scale=0.125, bias=padneg_sb[:, kb : kb + 1]
                )
                pts[kb] = pt

            def pv(kb):
                dd = kb - 4 * qc
                qoff = max(0, dd) * P
                pt = pts.pop(kb)
                for hh in range(2):
                    h = 2 * m + hh
                    nc.tensor.matmul(
                        pvs[hh][:, qoff:],
                        vaug_sb[:, kb, ds(h * (DK + 1), P)],
                        pt[:, hh * 512 + qoff : (hh + 1) * 512],
                        start=(kb == 0),
                        stop=(kb == nkb - 1),
                    )

            qk(0)
            for kb in range(1, nkb):
                qk(kb)
                pv(kb - 1)
                if mid_cb is not None:
                    mid_cb(kb - 1)
            pv(nkb - 1)
            if last_cb is not None:
                last_cb(pvs)
            if mid_cb is not None:
                mid_cb(nkb - 1)
            return pvs

        yts = {}

        def outproj_step(qc, si, nch):
            def step():
                s = qc * 4 + si
                if nch == 0:
                    yts[s] = ysb.tile([P, 1024], BF, name=f"yt{s}", tag="yt")
                yt = yts[s]
                yp = psY.tile([P, 512], F32, name=f"yp{s}_{nch}", tag="yp")
                for m in range(2):
                    nc.tensor.matmul(
                        yp[:],
                        ctx_sets[qc][m][:, ts(si, P)],
                        wo_sb[:, m, ds(nch * 512, 512)],
                        start=(m == 0),
                        stop=(m == 1),
                    )
                if nch == 0:
                    nc.scalar.copy(yt[:, ts(nch, 512)], yp[:])
                else:
                    nc.vector.tensor_copy(yt[:, ts(nch, 512)], yp[:])
                # spread the tail chunk's drain over both HWDGE queues
                q_eng = nc.scalar if (qc == NQC - 1 and nch == 1) else nc.sync
                q_eng.dma_start(
                    y_r[s][:, ds(nch * 512, 512)], yt[:, ts(nch, 512)]
                )

            return step

        def outproj(qc, sis):
            for si in sis:
                for nch in range(2):
                    outproj_step(qc, si, nch)()

        # Final chunk's output projection split at the pair accumulation:
        # the pair-0 partial matmuls depend only on flush(3,0), so they run
        # as filler inside unit(3,1); f32 partials park in SBUF and only the
        # pair-1 matmul + add remain after the final flush.  Bit-identical
        # to the fused accumulation (f32 sum, bf16 on the final write).
        ym = {}

        def tail_partial0_step(si, nch):
            def step():
                ym[(si, nch)] = const.tile(
                    [P, 512], F32, name=f"ym{si}_{nch}", tag=f"ym{si}_{nch}"
                )
                yp = psY.tile([P, 512], F32, name=f"yq{si}_{nch}", tag="yp")
                nc.tensor.matmul(
                    yp[:],
                    ctx_sets[NQC - 1][0][:, ts(si, P)],
                    wo_sb[:, 0, ds(nch * 512, 512)],
                    start=True,
                    stop=True,
                )
                if nch == 0:
                    nc.scalar.copy(ym[(si, nch)][:], yp[:])
                else:
                    nc.vector.tensor_copy(ym[(si, nch)][:], yp[:])

            return step

        def tail_final(si, nch):
            s = (NQC - 1) * 4 + si
            if nch == 0:
                yts[s] = ysb.tile([P, 1024], BF, name=f"yt{s}", tag="yt")
            yt = yts[s]
            yp = psY.tile([P, 512], F32, name=f"yr{si}_{nch}", tag="yp")
            nc.tensor.matmul(
                yp[:],
                ctx_sets[NQC - 1][1][:, ts(si, P)],
                wo_sb[:, 1, ds(nch * 512, 512)],
                start=True,
                stop=True,
            )
            nc.vector.tensor_add(yt[:, ts(nch, 512)], ym[(si, nch)][:], yp[:])
            q_eng = nc.scalar if nch == 1 else nc.sync
            q_eng.dma_start(y_r[s][:, ds(nch * 512, 512)], yt[:, ts(nch, 512)])

        # ---------------- interleaved schedule ----------------
        # Emission order IS the per-engine execution order.  Attention
        # key-blocks are the clock; all other PE work (projection chunks for
        # later qcs, the previous chunk's output projection, the deferred
        # normalization broadcasts) is queued as small filler steps and one
        # is popped after every key-block, so the PE always has ~1.5us of
        # work per ~1us of exp and never idles into a HAM re-throttle.
        from collections import deque

        filler = deque()

        def unit(qc, m):
            stride = 2 if qc >= 2 else 1

            def mid(kb):
                # kb0: filler; kb1: previous pair's broadcast+normalize (its
                # DVE chain then has ~2 key-blocks of runway and the PE
                # never waits on it); kb>=2: filler by stride
                if kb == 1:
                    flush_norm()
                elif (kb == 0 or kb % stride == 1 % stride) and filler:
                    filler.popleft()()

            scores_pair(
                qc, m, mid_cb=mid, last_cb=lambda pvs: start_norm(qc, m, pvs)
            )

        def queue_outproj(qc, sis):
            for si in sis:
                for nch in range(2):
                    filler.append(outproj_step(qc, si, nch))

        # chunks 0-1 as lumps under the input-DMA shadow: q/k steps first
        # (their weights land before wv), v steps after
        s0, s1 = proj_steps(0), proj_steps(1)
        for st in s0[:4] + s1[:4] + s0[4:] + s1[4:]:
            st()
        if "dbg_vaug" in ins:
            nc.gpsimd.dma_start(ins["dbg_vaug"][:], vaug_sb[:, 0, :])

        # attention chunks qc1/qc2 are PE-rich (they carry the projection
        # steps); output projections migrate as late as their flush allows
        # so the exp-rate-limited qc3 key-blocks all have PE filler
        filler.extend(proj_steps(2))
        unit(0, 0)
        unit(0, 1)
        if "dbg_ctx" in ins:
            nc.gpsimd.dma_start(ins["dbg_ctx"][:], ctx_sets[0][0][:])
        filler.extend(proj_steps(3))
        unit(1, 0)
        unit(1, 1)
        queue_outproj(0, (0, 1, 2, 3))
        unit(2, 0)
        queue_outproj(1, (0, 1))
        unit(2, 1)
        queue_outproj(1, (2, 3))
        queue_outproj(2, (0, 1))
        unit(3, 0)
        queue_outproj(2, (2, 3))
        for si in range(4):
            for nch in range(2):
                filler.append(tail_partial0_step(si, nch))
        unit(3, 1)
        while filler:
            filler.popleft()()
        flush_norm()
        for si in range(4):
            for nch in range(2):
                tail_final(si, nch)


# ---------------- host side ----------------

def _bf16(a):
    import ml_dtypes

    return np.asarray(a, dtype=np.float32).astype(ml_dtypes.bfloat16)


def make_in_maps(x, padding_mask, Wq, bq, Wk, Wv, Wo):
    """Build the 8 per-core input dicts from full inputs."""
    x = np.asarray(x, dtype=np.float32)
    pad = np.asarray(padding_mask)
    tri = np.where(
        np.arange(P)[:, None] > np.arange(P)[None, :], np.float32(NEG), np.float32(0)
    ).astype(np.float32)
    in_maps = []
    def swz_w(w):  # [D, 256] -> [P, KT, 256] partition-contiguous
        return np.ascontiguousarray(w.reshape(KT, P, 2 * P).transpose(1, 0, 2))

    for c in range(N_CORES):
        b, g = divmod(c, 4)
        R = slice(g * 256, g * 256 + 256)
        padneg = ((pad[b] == 0) * np.float32(PADBIAS)).reshape(ST, P).T.copy()
        xt4 = x[b].T.reshape(KT, P, NQC, 512).transpose(1, 2, 0, 3)
        in_maps.append(
            {
                "xt": _bf16(np.ascontiguousarray(xt4)),
                "wq": _bf16(swz_w(np.asarray(Wq, np.float32)[R, :].T)),
                "wk": _bf16(swz_w(np.asarray(Wk, np.float32)[R, :].T)),
                "wv": _bf16(swz_w(np.asarray(Wv, np.float32)[R, :].T)),
                "wo": _bf16(
                    np.ascontiguousarray(
                        np.asarray(Wo, np.float32)[:, R].T.reshape(2, P, D).transpose(
                            1, 0, 2
                        )
                    )
                ),
                "bq": np.ascontiguousarray(
                    np.asarray(bq, np.float32)[R].reshape(2, P).T
                ),
                "padneg": np.ascontiguousarray(padneg),
                "tri": tri,
            }
        )
    return in_maps


def postprocess(partials, x, padding_mask, Wv, bv, Wo, bo):
    """Sum per-core partials, add folded bias, fix fully-masked rows."""
    x = np.asarray(x, np.float32)
    pad = np.asarray(padding_mask)
    Wv = np.asarray(Wv, np.float32)
    bv = np.asarray(bv, np.float32)
    Wo = np.asarray(Wo, np.float32)
    bo = np.asarray(bo, np.float32)
    B = x.shape[0]
    y = np.zeros((B, S, D), dtype=np.float32)
    for c in range(N_CORES):
        y[c // 4] += np.asarray(partials[c], dtype=np.float32)
    y += (Wo @ bv + bo)[None, None, :]
    # fully-masked rows (reference: uniform attention over all keys)
    for b in range(B):
        nz = np.flatnonzero(pad[b] != 0)
        q0 = int(nz[0]) if len(nz) else S
        if q0 > 0:
            ctx_u = x[b].mean(axis=0) @ Wv.T + bv
            y[b, :q0, :] = ctx_u @ Wo.T + bo
    return y


_NC_CACHE = {}


def _get_program():
    if "nc" not in _NC_CACHE:
        _NC_CACHE["nc"] = build_program()
    return _NC_CACHE["nc"]


def kernel(
    x, padding_mask, Wq, bq, Wk, bk, Wv, bv, Wo, bo
):
    from concourse.bass_utils import run_bass_kernel_spmd

    nc = _get_program()
    in_maps = make_in_maps(x, padding_mask, Wq, bq, Wk, Wv, Wo)
    res = run_bass_kernel_spmd(nc, in_maps, core_ids=list(range(N_CORES)))
    partials = [res.results[c]["y"] for c in range(N_CORES)]
    return postprocess(partials, x, padding_mask, Wv, bv, Wo, bo)
